# revision 8
# baseline (speedup 1.0000x reference)
"""Bahdanau (additive) attention kernel for Trainium2, 8 NeuronCores.

Reference computation (per batch row b, with q = encoder_state[0, b]):
    u       = tanh(ctx @ W1 + (q @ W2) + b)          # (S, H)
    scores  = u @ w                                   # (S,)
    scores  = where(arange(S) < lens[b], scores, -inf)
    wts     = softmax(scores)  (== exp(scores)/sum since |scores| <= ||w||_1)
    out[b]  = wts @ ctx                               # (DV,)

Distribution: data-parallel over B=32 rows -> 8 cores x 4 row-slots.
Rows are snake-assigned by descending chunk count so per-slot chunk counts
(c_j, shared by all cores for SPMD) add minimal padding; padded chunks are
fully masked out on device, so correctness never depends on the schedule.

Device pipeline per 512-row context chunk:
  HBM --SWDGE cast f32->bf16--> nat[128p,4t,512d]
  PE transpose (16x 128x128)  -> ctxT[128d,4c,512s] (via PSUM, DVE/ACT copy)
  PE: uT[h,s] += W1_c^T ctxT_c  (bf16, f32 accum)
  ACT: u = tanh(uT + qb_row)    (PSUM->SBUF, bf16 out)
  PE: scores[s,1] = u^T w ; ACT exp -> e ; DVE mask (iota < len)
  PE: num[1,512] += e_m^T nat ; den via ones^T e_m
  slot end: out_row = num * (1/sum(den))
"""

import os
import sys
from contextlib import ExitStack

import numpy as np

sys.path.insert(0, "/opt/trn_rl_repo")

import concourse.bass as bass
import concourse.tile as tile
from concourse import mybir
from concourse.bass_utils import run_bass_kernel_spmd
from concourse.masks import make_identity

try:
    from ml_dtypes import bfloat16 as np_bf16
except ImportError:  # pragma: no cover
    import jax.numpy as jnp

    np_bf16 = jnp.bfloat16

B, S, DV, DQ, H = 32, 4096, 512, 512, 256
NCORES = 8
RPC = B // NCORES  # rows per core = 4
CH = 512  # context rows per chunk
P = 128

f32 = mybir.dt.float32
bf16 = mybir.dt.bfloat16
i32 = mybir.dt.int32
FT = mybir.ActivationFunctionType
ALU = mybir.AluOpType


def _ts(i, n):
    return slice(i * n, (i + 1) * n)


def _plan(lens: np.ndarray):
    """Snake-assign rows to (core, slot) minimizing padded chunk count."""
    cnt = np.maximum(1, -(-lens.astype(np.int64) // CH))  # ceil, >=1
    order = np.argsort(-cnt, kind="stable")
    assign = np.zeros((NCORES, RPC), dtype=np.int64)
    for j in range(RPC):
        idx = order[j * NCORES : (j + 1) * NCORES]
        if j % 2 == 1:
            idx = idx[::-1]
        assign[:, j] = idx
    cj = [int(cnt[assign[:, j]].max()) for j in range(RPC)]
    return assign, cj


def _build_program(cj):
    nc = bass.Bass()

    ctx_d = nc.declare_dram_parameter("ctx_local", [RPC, S, DV], f32, isOutput=False)
    q_d = nc.declare_dram_parameter("q_local", [RPC, DQ], bf16, isOutput=False)
    lens_d = nc.declare_dram_parameter("lens_local", [RPC], f32, isOutput=False)
    w1_d = nc.declare_dram_parameter("W1b", [DV, H], bf16, isOutput=False)
    w2_d = nc.declare_dram_parameter("W2b", [DQ, H], bf16, isOutput=False)
    w_d = nc.declare_dram_parameter("wb", [H, 1], bf16, isOutput=False)
    b_d = nc.declare_dram_parameter("bias_b", [H], f32, isOutput=False)
    out_d = nc.declare_dram_parameter("out_local", [RPC, DV], f32, isOutput=True)

    with tile.TileContext(nc) as tc, ExitStack() as ctx:
        consts = ctx.enter_context(tc.tile_pool(name="consts", bufs=1))
        nat_pool = ctx.enter_context(tc.tile_pool(name="nat", bufs=3))
        ctxT_pool = ctx.enter_context(tc.tile_pool(name="ctxT", bufs=2))
        ut_pool = ctx.enter_context(tc.tile_pool(name="ut", bufs=4))
        small = ctx.enter_context(tc.tile_pool(name="small", bufs=3))
        ps_tr = ctx.enter_context(tc.tile_pool(name="ps_tr", bufs=2, space="PSUM"))
        ps_ut = ctx.enter_context(tc.tile_pool(name="ps_ut", bufs=2, space="PSUM"))
        ps_sc = ctx.enter_context(tc.tile_pool(name="ps_sc", bufs=2, space="PSUM"))
        ps_num = ctx.enter_context(tc.tile_pool(name="ps_num", bufs=2, space="PSUM"))

        ident = consts.tile([P, P], bf16)
        make_identity(nc, ident)

        w1_sb = consts.tile([P, 4, H], bf16)
        nc.sync.dma_start(out=w1_sb, in_=w1_d.rearrange("(c p) h -> p c h", p=P))
        w2_sb = consts.tile([P, 4, H], bf16)
        nc.sync.dma_start(out=w2_sb, in_=w2_d.rearrange("(c p) h -> p c h", p=P))
        w_sb = consts.tile([P, 2], bf16)
        nc.sync.dma_start(out=w_sb, in_=w_d.rearrange("(c p) o -> p (c o)", p=P))
        b_sb = consts.tile([P, 2], f32)
        nc.sync.dma_start(out=b_sb, in_=b_d.rearrange("(c p) -> p c", p=P))
        ones_sb = consts.tile([P, 1], bf16)
        nc.vector.memset(ones_sb, 1.0)

        iota_i = consts.tile([P, S // P], i32)
        nc.gpsimd.iota(iota_i, pattern=[[P, S // P]], base=0, channel_multiplier=1)
        iota_f = consts.tile([P, S // P], f32)
        nc.vector.tensor_copy(iota_f, iota_i)

        lens_b = consts.tile([P, RPC], f32)
        lens_ap = lens_d[:]
        nc.gpsimd.dma_start(
            out=lens_b,
            in_=bass.AP(tensor=lens_ap.tensor, offset=lens_ap.offset,
                        ap=[[0, P]] + list(lens_ap.ap)),
        )
        lens_bc = consts.tile([P, RPC, 4], f32)
        for j in range(RPC):
            nc.vector.tensor_copy(
                lens_bc[:, j, :], lens_b[:, j : j + 1].broadcast_to([P, 4])
            )

        q_sb = consts.tile([RPC, DQ], bf16)
        nc.sync.dma_start(out=q_sb, in_=q_d[:, :])
        qT_sb = consts.tile([P, 4, RPC], bf16)
        for c in range(4):
            pst = ps_tr.tile([P, CH], bf16, tag="ps_tr")
            nc.tensor.transpose(pst[:, 0:RPC], q_sb[:, _ts(c, P)], ident[0:RPC, 0:RPC])
            nc.vector.tensor_copy(qT_sb[:, c, :], pst[:, 0:RPC])

        u_bias = consts.tile([P, 2, RPC], f32)
        for h in range(2):
            pqb = ps_sc.tile([P, 8], f32, tag="ps_sc")
            for c in range(4):
                nc.tensor.matmul(
                    pqb[:, 0:RPC],
                    lhsT=w2_sb[:, c, _ts(h, P)],
                    rhs=qT_sb[:, c, :],
                    start=(c == 0),
                    stop=(c == 3),
                )
            nc.vector.tensor_tensor(
                out=u_bias[:, h, :], in0=pqb[:, 0:RPC],
                in1=b_sb[:, h : h + 1].broadcast_to([P, RPC]), op=ALU.add,
            )

        for j in range(RPC):  # row slot
            num_ps = ps_num.tile([1, DV], f32)
            den_acc = small.tile([1, 4], f32, tag="den_acc")
            nchunks = cj[j]
            for k in range(nchunks):
                nat = nat_pool.tile([P, 4, CH], bf16, tag="nat")
                nc.gpsimd.dma_start(
                    out=nat,
                    in_=ctx_d[j, _ts(k, CH), :].rearrange("(t p) d -> p t d", p=P),
                )
                ctxT = ctxT_pool.tile([P, 4, CH], bf16, tag="ctxT")
                for c in range(4):
                    trp = ps_tr.tile([P, CH], bf16, tag="ps_tr")
                    for t in range(4):
                        nc.tensor.transpose(
                            trp[:, _ts(t, P)], nat[:, t, _ts(c, P)], ident
                        )
                    if c % 2 == 0:
                        nc.vector.tensor_copy(ctxT[:, c, :], trp)
                    else:
                        nc.scalar.activation(ctxT[:, c, :], trp, func=FT.Copy)

                ut_tiles = []
                for h in range(2):
                    put = ps_ut.tile([P, CH], f32, tag="ps_ut")
                    for c in range(4):
                        nc.tensor.matmul(
                            put,
                            lhsT=w1_sb[:, c, _ts(h, P)],
                            rhs=ctxT[:, c, :],
                            start=(c == 0),
                            stop=(c == 3),
                        )
                    ut = ut_pool.tile([P, CH], bf16, tag="ut")
                    nc.scalar.activation(
                        ut, put, func=FT.Tanh, bias=u_bias[:, h, j : j + 1]
                    )
                    ut_tiles.append(ut)

                sc = ps_sc.tile([P, 8], f32, tag="ps_sc")
                for t in range(4):
                    for h in range(2):
                        nc.tensor.matmul(
                            sc[:, t : t + 1],
                            lhsT=ut_tiles[h][:, _ts(t, P)],
                            rhs=w_sb[:, h : h + 1],
                            start=(h == 0),
                            stop=(h == 1),
                        )
                e_sb = small.tile([P, 4], bf16, tag="e_sb")
                nc.scalar.activation(e_sb, sc[:, 0:4], func=FT.Exp)
                mask = small.tile([P, 4], bf16, tag="mask")
                nc.vector.tensor_tensor(
                    out=mask, in0=iota_f[:, _ts(k, 4)],
                    in1=lens_bc[:, j, :], op=ALU.is_lt,
                )
                e_m = small.tile([P, 4], bf16, tag="e_m")
                nc.vector.tensor_tensor(out=e_m, in0=e_sb, in1=mask, op=ALU.mult)

                for t in range(4):
                    nc.tensor.matmul(
                        num_ps,
                        lhsT=e_m[:, t : t + 1],
                        rhs=nat[:, t, :],
                        start=(k == 0 and t == 0),
                        stop=(k == nchunks - 1 and t == 3),
                    )
                nc.tensor.matmul(
                    sc[0:1, 4:8], lhsT=ones_sb, rhs=e_m, start=True, stop=True
                )
                if k == 0:
                    nc.vector.tensor_copy(den_acc, sc[0:1, 4:8])
                else:
                    nc.vector.tensor_tensor(
                        out=den_acc, in0=den_acc, in1=sc[0:1, 4:8], op=ALU.add
                    )

            den_t = small.tile([1, 1], f32, tag="den_t")
            nc.vector.tensor_reduce(
                out=den_t, in_=den_acc, axis=mybir.AxisListType.X, op=ALU.add
            )
            rden = small.tile([1, 1], f32, tag="rden")
            nc.vector.reciprocal(rden, den_t)
            att = small.tile([1, DV], f32, tag="att")
            nc.vector.tensor_tensor(
                out=att, in0=num_ps,
                in1=rden[0:1, 0:1].broadcast_to([1, DV]), op=ALU.mult,
            )
            nc.sync.dma_start(out=out_d[j : j + 1, :], in_=att)

    return nc



_SPLIT_ENGINES = None  # computed lazily


def _split_multi_waits(nc):
    """Walrus in this container accepts at most one sync-wait per engine
    instruction. Move extra waits onto same-engine NoOps inserted just
    before the instruction (engines execute their stream in order)."""
    eng_ok = {
        mybir.EngineType.PE,
        mybir.EngineType.DVE,
        mybir.EngineType.Activation,
        mybir.EngineType.Pool,
        mybir.EngineType.SP,
    }
    nid = [0]
    for f in nc.m.functions:
        for blk in f.blocks:
            new_insts = []
            for inst in blk.instructions:
                si = getattr(inst, "sync_info", None)
                if (
                    si is not None
                    and si.on_wait
                    and len(si.on_wait) > 1
                    and getattr(inst, "engine", None) in eng_ok
                ):
                    extra, keep = si.on_wait[:-1], si.on_wait[-1:]
                    for wsub in extra:
                        nop = mybir.InstNoOp(
                            name=f"I-waitsplit-{nid[0]}", ins=[], outs=[]
                        )
                        nid[0] += 1
                        nop.engine = inst.engine
                        nop.sync_info = mybir.SyncInfo(
                            on_wait=[wsub], on_update=[]
                        )
                        new_insts.append(nop)
                    si.on_wait = keep
                new_insts.append(inst)
            blk.instructions[:] = new_insts
    return nc


def _ensure_ntff_hook():
    """bass_utils wants antenv.axon_hooks for trace=True under axon; the
    image lacks it. Provide a shim wired to the boot module's ctypes hook."""
    import types
    try:
        import antenv.axon_hooks  # noqa: F401
        return True
    except ImportError:
        pass
    try:
        sys.path.insert(0, "/root/.axon_site")
        from trn_agent_boot.trn_boot import _ntff_profile_via_ctypes
        hook = _ntff_profile_via_ctypes("/opt/axon/libaxon_pjrt.so")
        if hook is None:
            return False
        mod = types.ModuleType("antenv.axon_hooks")
        _h = [hook]
        mod.set_axon_ntff_profile_hook = lambda h: _h.__setitem__(0, h)
        mod.get_axon_ntff_profile_hook = lambda: _h[0]
        sys.modules["antenv.axon_hooks"] = mod
        import antenv
        antenv.axon_hooks = mod
        return True
    except Exception as e:  # pragma: no cover
        print("ntff hook shim failed:", e)
        return False

_RESULT_CACHE = {}


def kernel(encoder_state, context, lens, W1, W2, b, w, _want_time=False):
    assign, cj = _plan(np.asarray(lens))

    nc = _split_multi_waits(_build_program(cj))

    w1b = np.asarray(W1, dtype=np.float32).astype(np_bf16)
    w2b = np.asarray(W2, dtype=np.float32).astype(np_bf16)
    wb = np.asarray(w, dtype=np.float32).astype(np_bf16)
    bias_b = np.asarray(b, dtype=np.float32).reshape(H)
    q_full = np.asarray(encoder_state, dtype=np.float32)[0]  # (B, DQ)
    ctx_full = np.ascontiguousarray(np.asarray(context, dtype=np.float32))
    lens_f = np.asarray(lens).astype(np.float32)

    in_maps = []
    for i in range(NCORES):
        rows = assign[i]
        in_maps.append(
            {
                "ctx_local": np.ascontiguousarray(ctx_full[rows]),
                "q_local": np.ascontiguousarray(q_full[rows]).astype(np_bf16),
                "lens_local": np.ascontiguousarray(lens_f[rows]),
                "W1b": w1b,
                "W2b": w2b,
                "wb": wb,
                "bias_b": bias_b,
            }
        )

    do_trace = bool(_want_time) and _ensure_ntff_hook()
    res = run_bass_kernel_spmd(
        nc, in_maps, list(range(NCORES)), trace=do_trace
    )

    out = np.zeros((B, DV), dtype=np.float32)
    for i in range(NCORES):
        loc = res.results[i]["out_local"]
        for j in range(RPC):
            out[assign[i, j]] = loc[j]

    _RESULT_CACHE["exec_time_ns"] = res.exec_time_ns
    _RESULT_CACHE["mean_exec_time_ns"] = res.mean_exec_time_ns
    return out


# revision 9
# speedup vs baseline: 1.0069x; 1.0069x over previous
"""Bahdanau (additive) attention kernel for Trainium2, 8 NeuronCores.

Reference computation (per batch row b, with q = encoder_state[0, b]):
    u       = tanh(ctx @ W1 + (q @ W2) + b)          # (S, H)
    scores  = u @ w                                   # (S,)
    scores  = where(arange(S) < lens[b], scores, -inf)
    wts     = softmax(scores)  (== exp(scores)/sum since |scores| <= ||w||_1)
    out[b]  = wts @ ctx                               # (DV,)

Distribution: data-parallel over B=32 rows -> 8 cores x 4 row-slots.
Rows are snake-assigned by descending chunk count so per-slot chunk counts
(c_j, shared by all cores for SPMD) add minimal padding; padded chunks are
fully masked out on device, so correctness never depends on the schedule.

Device pipeline per 512-row context chunk:
  HBM --SWDGE cast f32->bf16--> nat[128p,4t,512d]
  PE transpose (16x 128x128)  -> ctxT[128d,4c,512s] (via PSUM, DVE/ACT copy)
  PE: uT[h,s] += W1_c^T ctxT_c  (bf16, f32 accum)
  ACT: u = tanh(uT + qb_row)    (PSUM->SBUF, bf16 out)
  PE: scores[s,1] = u^T w ; ACT exp -> e ; DVE mask (iota < len)
  PE: num[1,512] += e_m^T nat ; den via ones^T e_m
  slot end: out_row = num * (1/sum(den))
"""

import os
import sys
from contextlib import ExitStack

import numpy as np

sys.path.insert(0, "/opt/trn_rl_repo")

import concourse.bass as bass
import concourse.tile as tile
from concourse import mybir
from concourse.bass_utils import run_bass_kernel_spmd
from concourse.masks import make_identity

try:
    from ml_dtypes import bfloat16 as np_bf16
except ImportError:  # pragma: no cover
    import jax.numpy as jnp

    np_bf16 = jnp.bfloat16

B, S, DV, DQ, H = 32, 4096, 512, 512, 256
NCORES = 8
RPC = B // NCORES  # rows per core = 4
CH = 512  # context rows per chunk
P = 128

f32 = mybir.dt.float32
bf16 = mybir.dt.bfloat16
i32 = mybir.dt.int32
FT = mybir.ActivationFunctionType
ALU = mybir.AluOpType


def _ts(i, n):
    return slice(i * n, (i + 1) * n)


def _plan(lens: np.ndarray):
    """Snake-assign rows to (core, slot) minimizing padded chunk count."""
    cnt = np.maximum(1, -(-lens.astype(np.int64) // CH))  # ceil, >=1
    order = np.argsort(-cnt, kind="stable")
    assign = np.zeros((NCORES, RPC), dtype=np.int64)
    for j in range(RPC):
        idx = order[j * NCORES : (j + 1) * NCORES]
        if j % 2 == 1:
            idx = idx[::-1]
        assign[:, j] = idx
    cj = [int(cnt[assign[:, j]].max()) for j in range(RPC)]
    return assign, cj


def _build_program(cj):
    nc = bass.Bass()

    ctx_d = nc.declare_dram_parameter("ctx_local", [RPC, S, DV], f32, isOutput=False)
    q_d = nc.declare_dram_parameter("q_local", [RPC, DQ], bf16, isOutput=False)
    lens_d = nc.declare_dram_parameter("lens_local", [RPC], f32, isOutput=False)
    w1_d = nc.declare_dram_parameter("W1b", [DV, H], bf16, isOutput=False)
    w2_d = nc.declare_dram_parameter("W2b", [DQ, H], bf16, isOutput=False)
    w_d = nc.declare_dram_parameter("wb", [H, 1], bf16, isOutput=False)
    b_d = nc.declare_dram_parameter("bias_b", [H], f32, isOutput=False)
    out_d = nc.declare_dram_parameter("out_local", [RPC, DV], f32, isOutput=True)

    with tile.TileContext(nc) as tc, ExitStack() as ctx:
        consts = ctx.enter_context(tc.tile_pool(name="consts", bufs=1))
        nat_pool = ctx.enter_context(tc.tile_pool(name="nat", bufs=2))
        ctxT_pool = ctx.enter_context(tc.tile_pool(name="ctxT", bufs=2))
        ut_pool = ctx.enter_context(tc.tile_pool(name="ut", bufs=4))
        small = ctx.enter_context(tc.tile_pool(name="small", bufs=3))
        ps_tr = ctx.enter_context(tc.tile_pool(name="ps_tr", bufs=2, space="PSUM"))
        ps_ut = ctx.enter_context(tc.tile_pool(name="ps_ut", bufs=3, space="PSUM"))
        ps_sc = ctx.enter_context(tc.tile_pool(name="ps_sc", bufs=2, space="PSUM"))
        ps_num = ctx.enter_context(tc.tile_pool(name="ps_num", bufs=1, space="PSUM"))

        ident = consts.tile([P, P], bf16)
        make_identity(nc, ident)

        w1_sb = consts.tile([P, 4, H], bf16)
        nc.sync.dma_start(out=w1_sb, in_=w1_d.rearrange("(c p) h -> p c h", p=P))
        w2_sb = consts.tile([P, 4, H], bf16)
        nc.sync.dma_start(out=w2_sb, in_=w2_d.rearrange("(c p) h -> p c h", p=P))
        w_sb = consts.tile([P, 2], bf16)
        nc.sync.dma_start(out=w_sb, in_=w_d.rearrange("(c p) o -> p (c o)", p=P))
        b_sb = consts.tile([P, 2], f32)
        nc.sync.dma_start(out=b_sb, in_=b_d.rearrange("(c p) -> p c", p=P))
        ones_sb = consts.tile([P, 1], bf16)
        nc.vector.memset(ones_sb, 1.0)

        iota_i = consts.tile([P, S // P], i32)
        nc.gpsimd.iota(iota_i, pattern=[[P, S // P]], base=0, channel_multiplier=1)
        iota_f = consts.tile([P, S // P], f32)
        nc.vector.tensor_copy(iota_f, iota_i)

        lens_b = consts.tile([P, RPC], f32)
        lens_ap = lens_d[:]
        nc.gpsimd.dma_start(
            out=lens_b,
            in_=bass.AP(tensor=lens_ap.tensor, offset=lens_ap.offset,
                        ap=[[0, P]] + list(lens_ap.ap)),
        )
        lens_bc = consts.tile([P, RPC, 4], f32)
        for j in range(RPC):
            nc.vector.tensor_copy(
                lens_bc[:, j, :], lens_b[:, j : j + 1].broadcast_to([P, 4])
            )

        q_sb = consts.tile([RPC, DQ], bf16)
        nc.sync.dma_start(out=q_sb, in_=q_d[:, :])
        qT_sb = consts.tile([P, 4, RPC], bf16)
        for c in range(4):
            pst = ps_tr.tile([P, CH], bf16, tag="ps_tr")
            nc.tensor.transpose(pst[:, 0:RPC], q_sb[:, _ts(c, P)], ident[0:RPC, 0:RPC])
            nc.vector.tensor_copy(qT_sb[:, c, :], pst[:, 0:RPC])

        u_bias = consts.tile([P, 2, RPC], f32)
        for h in range(2):
            pqb = ps_sc.tile([P, 8], f32, tag="ps_sc")
            for c in range(4):
                nc.tensor.matmul(
                    pqb[:, 0:RPC],
                    lhsT=w2_sb[:, c, _ts(h, P)],
                    rhs=qT_sb[:, c, :],
                    start=(c == 0),
                    stop=(c == 3),
                )
            nc.vector.tensor_tensor(
                out=u_bias[:, h, :], in0=pqb[:, 0:RPC],
                in1=b_sb[:, h : h + 1].broadcast_to([P, RPC]), op=ALU.add,
            )

        for j in range(RPC):  # row slot
            num_ps = ps_num.tile([1, DV], f32)
            den_acc = small.tile([1, 4], f32, tag="den_acc")
            nchunks = cj[j]
            nat_slot = nat_pool.tile([P, 4 * nchunks, CH], bf16, tag="nat")
            nc.gpsimd.dma_start(
                out=nat_slot,
                in_=ctx_d[j, 0 : nchunks * CH, :].rearrange(
                    "(t p) d -> p t d", p=P
                ),
            )
            for k in range(nchunks):
                nat = nat_slot[:, 4 * k : 4 * k + 4, :]
                ctxT = ctxT_pool.tile([P, 4, CH], bf16, tag="ctxT")
                for c in range(4):
                    trp = ps_tr.tile([P, CH], bf16, tag="ps_tr")
                    for t in range(4):
                        nc.tensor.transpose(
                            trp[:, _ts(t, P)], nat[:, t, _ts(c, P)], ident
                        )
                    if c % 2 == 0:
                        nc.vector.tensor_copy(ctxT[:, c, :], trp)
                    else:
                        nc.scalar.activation(ctxT[:, c, :], trp, func=FT.Copy)

                ut_tiles = []
                for h in range(2):
                    put = ps_ut.tile([P, CH], f32, tag="ps_ut")
                    for c in range(4):
                        nc.tensor.matmul(
                            put,
                            lhsT=w1_sb[:, c, _ts(h, P)],
                            rhs=ctxT[:, c, :],
                            start=(c == 0),
                            stop=(c == 3),
                        )
                    ut = ut_pool.tile([P, CH], bf16, tag="ut")
                    nc.scalar.activation(
                        ut, put, func=FT.Tanh, bias=u_bias[:, h, j : j + 1]
                    )
                    ut_tiles.append(ut)

                sc = ps_sc.tile([P, 8], f32, tag="ps_sc")
                for t in range(4):
                    for h in range(2):
                        nc.tensor.matmul(
                            sc[:, t : t + 1],
                            lhsT=ut_tiles[h][:, _ts(t, P)],
                            rhs=w_sb[:, h : h + 1],
                            start=(h == 0),
                            stop=(h == 1),
                        )
                e_sb = small.tile([P, 4], bf16, tag="e_sb")
                nc.scalar.activation(e_sb, sc[:, 0:4], func=FT.Exp)
                mask = small.tile([P, 4], bf16, tag="mask")
                nc.vector.tensor_tensor(
                    out=mask, in0=iota_f[:, _ts(k, 4)],
                    in1=lens_bc[:, j, :], op=ALU.is_lt,
                )
                e_m = small.tile([P, 4], bf16, tag="e_m")
                nc.vector.tensor_tensor(out=e_m, in0=e_sb, in1=mask, op=ALU.mult)

                for t in range(4):
                    nc.tensor.matmul(
                        num_ps,
                        lhsT=e_m[:, t : t + 1],
                        rhs=nat[:, t, :],
                        start=(k == 0 and t == 0),
                        stop=(k == nchunks - 1 and t == 3),
                    )
                nc.tensor.matmul(
                    sc[0:1, 4:8], lhsT=ones_sb, rhs=e_m, start=True, stop=True
                )
                if k == 0:
                    nc.vector.tensor_copy(den_acc, sc[0:1, 4:8])
                else:
                    nc.vector.tensor_tensor(
                        out=den_acc, in0=den_acc, in1=sc[0:1, 4:8], op=ALU.add
                    )

            den_t = small.tile([1, 1], f32, tag="den_t")
            nc.vector.tensor_reduce(
                out=den_t, in_=den_acc, axis=mybir.AxisListType.X, op=ALU.add
            )
            rden = small.tile([1, 1], f32, tag="rden")
            nc.vector.reciprocal(rden, den_t)
            att = small.tile([1, DV], f32, tag="att")
            nc.vector.tensor_tensor(
                out=att, in0=num_ps,
                in1=rden[0:1, 0:1].broadcast_to([1, DV]), op=ALU.mult,
            )
            nc.sync.dma_start(out=out_d[j : j + 1, :], in_=att)

    return nc



_SPLIT_ENGINES = None  # computed lazily


def _split_multi_waits(nc):
    """Walrus in this container accepts at most one sync-wait per engine
    instruction. Move extra waits onto same-engine NoOps inserted just
    before the instruction (engines execute their stream in order)."""
    eng_ok = {
        mybir.EngineType.PE,
        mybir.EngineType.DVE,
        mybir.EngineType.Activation,
        mybir.EngineType.Pool,
        mybir.EngineType.SP,
    }
    nid = [0]
    for f in nc.m.functions:
        for blk in f.blocks:
            new_insts = []
            for inst in blk.instructions:
                si = getattr(inst, "sync_info", None)
                if (
                    si is not None
                    and si.on_wait
                    and len(si.on_wait) > 1
                    and getattr(inst, "engine", None) in eng_ok
                ):
                    extra, keep = si.on_wait[:-1], si.on_wait[-1:]
                    for wsub in extra:
                        nop = mybir.InstNoOp(
                            name=f"I-waitsplit-{nid[0]}", ins=[], outs=[]
                        )
                        nid[0] += 1
                        nop.engine = inst.engine
                        nop.sync_info = mybir.SyncInfo(
                            on_wait=[wsub], on_update=[]
                        )
                        new_insts.append(nop)
                    si.on_wait = keep
                new_insts.append(inst)
            blk.instructions[:] = new_insts
    return nc


def _ensure_ntff_hook():
    """bass_utils wants antenv.axon_hooks for trace=True under axon; the
    image lacks it. Provide a shim wired to the boot module's ctypes hook."""
    import types
    try:
        import antenv.axon_hooks  # noqa: F401
        return True
    except ImportError:
        pass
    try:
        sys.path.insert(0, "/root/.axon_site")
        from trn_agent_boot.trn_boot import _ntff_profile_via_ctypes
        hook = _ntff_profile_via_ctypes("/opt/axon/libaxon_pjrt.so")
        if hook is None:
            return False
        mod = types.ModuleType("antenv.axon_hooks")
        _h = [hook]
        mod.set_axon_ntff_profile_hook = lambda h: _h.__setitem__(0, h)
        mod.get_axon_ntff_profile_hook = lambda: _h[0]
        sys.modules["antenv.axon_hooks"] = mod
        import antenv
        antenv.axon_hooks = mod
        return True
    except Exception as e:  # pragma: no cover
        print("ntff hook shim failed:", e)
        return False

_RESULT_CACHE = {}


def kernel(encoder_state, context, lens, W1, W2, b, w, _want_time=False):
    assign, cj = _plan(np.asarray(lens))

    nc = _split_multi_waits(_build_program(cj))

    w1b = np.asarray(W1, dtype=np.float32).astype(np_bf16)
    w2b = np.asarray(W2, dtype=np.float32).astype(np_bf16)
    wb = np.asarray(w, dtype=np.float32).astype(np_bf16)
    bias_b = np.asarray(b, dtype=np.float32).reshape(H)
    q_full = np.asarray(encoder_state, dtype=np.float32)[0]  # (B, DQ)
    ctx_full = np.ascontiguousarray(np.asarray(context, dtype=np.float32))
    lens_f = np.asarray(lens).astype(np.float32)

    in_maps = []
    for i in range(NCORES):
        rows = assign[i]
        in_maps.append(
            {
                "ctx_local": np.ascontiguousarray(ctx_full[rows]),
                "q_local": np.ascontiguousarray(q_full[rows]).astype(np_bf16),
                "lens_local": np.ascontiguousarray(lens_f[rows]),
                "W1b": w1b,
                "W2b": w2b,
                "wb": wb,
                "bias_b": bias_b,
            }
        )

    do_trace = bool(_want_time) and _ensure_ntff_hook()
    res = run_bass_kernel_spmd(
        nc, in_maps, list(range(NCORES)), trace=do_trace
    )

    out = np.zeros((B, DV), dtype=np.float32)
    for i in range(NCORES):
        loc = res.results[i]["out_local"]
        for j in range(RPC):
            out[assign[i, j]] = loc[j]

    _RESULT_CACHE["exec_time_ns"] = res.exec_time_ns
    _RESULT_CACHE["mean_exec_time_ns"] = res.mean_exec_time_ns
    return out
